# revision 23
# baseline (speedup 1.0000x reference)
"""Distributed SAGE GNN kernel for 8 TRN2 NeuronCores.

Strategy (per sharding hint): nodes and their output rows are sharded across
the 8 cores; edges are partitioned by destination core. Weights replicated.

Per layer:
  1. pre-transform: y = input @ W_l.T computed on each core's node shard
     (feature-major input tiles as matmul lhsT), then one AllGather so every
     core holds the full y (message table) in its HBM.
  2. aggregation: for each 128-node destination tile, gather y[src] rows with
     dma_gather (bf16 rows, int16 window-local indices, 4 source windows of
     25088 rows so indices fit int16; 4 SWDGE queues round-robin) and
     accumulate  agg[n, f] = sum_e onehot[e, n] * y[src_e, f]  as one-hot x
     message matmuls in PSUM. One-hots are built 16 pairs per DVE op via
     broadcast is_equal against an iota tile; 128-edge chunks may straddle
     tile boundaries (one matmul per (chunk, tile) pair, the dst-relative
     meta column masks foreign edges).
  3. finalize: h^T = relu(W_r.T-term + bias + agg/deg) fused into one PSUM
     accumulation group (bias via a K=1 matmul, agg via a transposing matmul
     with lhsT=agg), the 1/deg mean scale applied by the Scalar engine on the
     PSUM->SBUF copy. Layer 2's pre-transform is fused into layer 1's
     finalize; actor/critic heads are tiny matmuls/reductions in the L2 loop.
"""

import numpy as np
import ml_dtypes

import concourse.bass as bass
import concourse.mybir as mybir
import concourse.tile as tile
from concourse import bacc
from concourse.bass_utils import run_bass_kernel_spmd

BF16 = ml_dtypes.bfloat16

N = 100000
E = 1600000
D = 128
NCORES = 8
VPC = 12544            # nodes per core (padded)
NPAD = VPC * NCORES    # 100352
TPC = VPC // 128       # 98 dst tiles per core
NSRC = 4               # gather source windows (int16 index limit)
SRCW = NPAD // NSRC    # 25088
TB = 3                 # dst tiles per block (PSUM-resident accumulators)
MAXCH = 32             # max 128-edge chunks per dma_gather call
OHG = 16               # pairs per batched one-hot DVE op

# wbf plane column layout (bf16)
W1L, W1R, W2L, W2R = 0, 128, 256, 384
WACOL = 512
IOTA, IDENT = 514, 642
ONESROW, B1ROW, B2ROW = 770, 898, 1026
WBFW = 1154
# fb plane (f32): [0,0]=ba, col1 = Wc, cols 2.. = 1/deg per tile
FBW = 2 + TPC

LAST_RESULT = None     # BassKernelResults of the most recent run (for tests)


class _Call:
    __slots__ = ("s", "nch", "icol", "chunks", "tailpad")

    def __init__(self, s, icol):
        self.s = s
        self.nch = 0
        self.icol = icol      # column offset into the idx plane
        self.chunks = []      # list of (t, pair_col, first, last, chunk_in_call)
        self.tailpad = 0      # trailing pad slots (gather row 0, masked)


def _build_schedule(edge_index):
    """Host-side edge partitioning. Returns (sched, per-core planes).

    Edge stream layout per core (identical shape on every core): for each
    block of TB dst tiles, for each source window s, the edges of groups
    (s, t in blk) are packed back-to-back, each group padded to the max count
    over cores (pads: idx 0, sentinel dst). The packed segment is rounded up
    to a multiple of 128. 128-edge chunks may straddle tile boundaries; each
    (chunk, tile) pair becomes one one-hot matmul whose dst-relative meta
    column masks out the other tiles' edges.
    """
    src = np.asarray(edge_index[0], dtype=np.int64)
    dst = np.asarray(edge_index[1], dtype=np.int64)

    core = dst // VPC
    t_loc = (dst - core * VPC) // 128
    s_chunk = src // SRCW
    grp = (core * NSRC + s_chunk) * TPC + t_loc          # [E]

    cnt = np.bincount(grp, minlength=NCORES * NSRC * TPC)
    cnt = cnt.reshape(NCORES, NSRC, TPC)
    C = cnt.max(axis=0)                                  # exact per (s, t)

    deg = np.bincount(dst, minlength=NPAD).astype(np.float64)
    invdeg = (1.0 / np.maximum(deg, 1.0)).astype(np.float32)   # [NPAD]

    blocks = [list(range(b, min(b + TB, TPC))) for b in range(0, TPC, TB)]

    slot_base = np.zeros((NSRC, TPC), dtype=np.int64)
    seg_meta = []            # per (blk, s): (slot0, seg_len, pad_len, groups)
    off = 0
    for blk in blocks:
        for s in range(NSRC):
            g0 = off
            groups = []
            for t in blk:
                slot_base[s, t] = off
                groups.append((t, off - g0, int(C[s, t])))
                off += int(C[s, t])
            seg_len = off - g0
            pad = (-seg_len) % 128
            off += pad
            seg_meta.append((g0, seg_len, pad, groups))
    etot = off
    assert etot % 128 == 0

    # total (chunk, tile) pairs per tile, for start/stop flags
    t_total = np.zeros(TPC, dtype=np.int64)
    for (g0, seg_len, pad, groups) in seg_meta:
        nch_seg = (seg_len + pad) // 128
        for k in range(nch_seg):
            a, b = k * 128, k * 128 + 128
            for (t, goff, glen) in groups:
                if glen and goff < b and a < goff + glen:
                    t_total[t] += 1

    calls_by_blk = [[] for _ in blocks]
    t_seen = np.zeros(TPC, dtype=np.int64)
    pc = 0
    icol = 0
    seg_i = 0
    pair_info = []               # (slot0_of_chunk, t) per pair
    for bi, blk in enumerate(blocks):
        calls = calls_by_blk[bi]
        for s in range(NSRC):
            g0, seg_len, pad, groups = seg_meta[seg_i]
            seg_i += 1
            nch_seg = (seg_len + pad) // 128
            call = None
            for k in range(nch_seg):
                if call is None or call.nch == MAXCH:
                    if call is not None:
                        calls.append(call)
                    call = _Call(s, icol)
                a, b = k * 128, k * 128 + 128
                j_in_call = call.nch
                for (t, goff, glen) in groups:
                    if glen and goff < b and a < goff + glen:
                        first = t_seen[t] == 0
                        t_seen[t] += 1
                        last = t_seen[t] == t_total[t]
                        call.chunks.append((t, pc, bool(first), bool(last), j_in_call))
                        pair_info.append((g0 + a, t))
                        pc += 1
                call.nch += 1
                icol += 8
            call.tailpad = pad
            calls.append(call)
            call = None
    n_pairs = pc
    idx_cols = icol

    # ---- per-core slot arrays -------------------------------------------
    order = np.argsort(grp, kind="stable")
    sg = grp[order]
    n_groups = NCORES * NSRC * TPC
    gstart = np.searchsorted(sg, np.arange(n_groups))
    rank = np.arange(E, dtype=np.int64) - gstart[sg]
    s_o = s_chunk[order]
    t_o = t_loc[order]
    core_o = core[order]
    src_o = src[order]
    dst_o = dst[order]
    slotpos = slot_base[s_o, t_o] + rank

    planes = []
    for p in range(NCORES):
        m = core_o == p
        sl = slotpos[m]
        s_src = np.zeros(etot, dtype=np.int16)
        s_dst = np.full(etot, 999.0, dtype=np.float32)
        s_tile = np.full(etot, -1, dtype=np.int64)
        s_src[sl] = (src_o[m] - s_o[m] * SRCW).astype(np.int16)
        s_dst[sl] = (dst_o[m] - (p * VPC + t_o[m] * 128)).astype(np.float32)
        s_tile[sl] = t_o[m]

        idx_plane = np.zeros((16, idx_cols), dtype=np.int16)
        meta = np.full((128, n_pairs), 999.0, dtype=np.float32)
        for calls in calls_by_blk:
            for call in calls:
                nidx = call.nch * 128
                base_slot = None
                for (t, pcx, first, last, j_in_call) in call.chunks:
                    if base_slot is None:
                        base_slot = pair_info[pcx][0] - j_in_call * 128
                    a = pair_info[pcx][0]
                    col = s_dst[a:a + 128].copy()
                    col[s_tile[a:a + 128] != t] = 999.0
                    meta[:, pcx] = col
                arr = s_src[base_slot: base_slot + nidx]
                idx_plane[:, call.icol: call.icol + nidx // 16] = (
                    arr.reshape(-1, 16).T
                )
        idx_plane = np.tile(idx_plane, (8, 1))

        inv_p = invdeg[p * VPC:(p + 1) * VPC].reshape(TPC, 128).T  # [128, TPC]
        planes.append((idx_plane, meta.astype(BF16), np.ascontiguousarray(inv_p)))

    sched = {
        "blocks": blocks,
        "calls_by_blk": calls_by_blk,
        "n_chunks": n_pairs,
        "idx_cols": idx_cols,
        "etot": etot,
        "max_nch": max(c.nch for calls in calls_by_blk for c in calls),
    }
    return sched, planes


def _build_kernel(sched):
    nc = bacc.Bacc(num_devices=NCORES, num_swdge_queues=4)
    bf = mybir.dt.bfloat16
    f32 = mybir.dt.float32
    n_chunks = sched["n_chunks"]
    idx_cols = sched["idx_cols"]
    max_nch = sched["max_nch"]

    xt_ext = nc.declare_dram_parameter("xt", [D, VPC], bf, isOutput=False)
    gidx_ext = nc.declare_dram_parameter("gidx", [128, idx_cols], mybir.dt.int16, isOutput=False)
    meta_ext = nc.declare_dram_parameter("meta", [128, n_chunks], bf, isOutput=False)
    wbf_ext = nc.declare_dram_parameter("wbf", [128, WBFW], bf, isOutput=False)
    fb_ext = nc.declare_dram_parameter("fb", [128, FBW], f32, isOutput=False)
    actor_ext = nc.declare_dram_parameter("actor", [1, VPC], f32, isOutput=True)
    crit_ext = nc.declare_dram_parameter("crit", [1, 1], f32, isOutput=True)

    with tile.TileContext(nc) as tc:
        with (
            tc.tile_pool(name="cpool", bufs=1) as cpool,
            tc.tile_pool(name="gpool", bufs=12) as gpool,
            tc.tile_pool(name="ohpool", bufs=12) as ohpool,
            tc.tile_pool(name="hpool", bufs=4) as hpool,
            tc.tile_pool(name="agg_psum", bufs=6, space="PSUM") as agg_psum,
            tc.tile_pool(name="mm_psum", bufs=2, space="PSUM") as mm_psum,
            tc.tile_pool(name="dram", bufs=1, space="DRAM") as dram,
        ):
            gidx_sb = cpool.tile([128, idx_cols], mybir.dt.int16)
            nc.sync.dma_start(out=gidx_sb[:], in_=gidx_ext[:])
            meta_sb = cpool.tile([128, n_chunks], bf)
            nc.sync.dma_start(out=meta_sb[:], in_=meta_ext[:])
            wbf_sb = cpool.tile([128, WBFW], bf)
            nc.sync.dma_start(out=wbf_sb[:], in_=wbf_ext[:])
            fb_sb = cpool.tile([128, FBW], f32)
            nc.sync.dma_start(out=fb_sb[:], in_=fb_ext[:])
            xt_sb = cpool.tile([128, VPC], bf)
            nc.sync.dma_start(out=xt_sb[:], in_=xt_ext[:])
            h1t_sb = cpool.tile([128, VPC], bf)

            iota = wbf_sb[:, IOTA:IOTA + 128]
            ident = wbf_sb[:, IDENT:IDENT + 128]
            ones_row = wbf_sb[0:1, ONESROW:ONESROW + 128]

            crit_acc = cpool.tile([128, 1], f32)
            nc.vector.memset(crit_acc[:], 0.0)

            y_local = dram.tile([VPC, D], bf)
            y_full0 = dram.tile([NPAD, D], bf, addr_space="Shared")
            y_full1 = dram.tile([NPAD, D], bf, addr_space="Shared")

            qctr = [0]
            for L in range(2):
                y_full = y_full0 if L == 0 else y_full1
                srcT = xt_sb if L == 0 else h1t_sb
                wl = wbf_sb[:, (W1L if L == 0 else W2L):(W1L if L == 0 else W2L) + 128]
                wr = wbf_sb[:, (W1R if L == 0 else W2R):(W1R if L == 0 else W2R) + 128]
                b_row = wbf_sb[0:1, (B1ROW if L == 0 else B2ROW):(B1ROW if L == 0 else B2ROW) + 128]

                # ---- pre-transform (L1's is fused into L0's finalize below)
                if L == 0:
                    for t in range(TPC):
                        yp = mm_psum.tile([128, 128], f32, tag="mm", name=f"yp_{L}_{t}")
                        nc.tensor.matmul(
                            out=yp[:], lhsT=srcT[:, t * 128:(t + 1) * 128], rhs=wl,
                            start=True, stop=True,
                        )
                        yl_sb = hpool.tile([128, 128], bf, tag="ylsb", name=f"ylsb_{L}_{t}")
                        nc.scalar.copy(out=yl_sb[:], in_=yp[:])
                        nc.sync.dma_start(out=y_local[t * 128:(t + 1) * 128, :], in_=yl_sb[:])

                nc.gpsimd.collective_compute(
                    "AllGather",
                    mybir.AluOpType.bypass,
                    replica_groups=[list(range(NCORES))],
                    ins=[y_local.opt()],
                    outs=[y_full.opt()],
                )

                # ---- aggregation + finalize, per destination-tile block
                for bi, blk in enumerate(sched["blocks"]):
                    aggs = {}
                    for t in blk:
                        aggs[t] = agg_psum.tile([128, 128], f32, tag="agg", name=f"agg_{L}_{t}")
                    for ci, call in enumerate(sched["calls_by_blk"][bi]):
                        g = gpool.tile([128, max_nch, 128], bf, tag="g", name=f"g_{L}_{bi}_{call.icol}")
                        nidx = call.nch * 128
                        nc.gpsimd.dma_gather(
                            out_ap=g[:, :call.nch, :],
                            in_ap=y_full[call.s * SRCW:(call.s + 1) * SRCW, :],
                            idxs_ap=gidx_sb[:, call.icol: call.icol + nidx // 16],
                            num_idxs=nidx,
                            num_idxs_reg=nidx,
                            elem_size=D,
                            single_packet=False,
                            queue_num=qctr[0] % 4,
                        )
                        qctr[0] += 1
                        npair = len(call.chunks)
                        for j0 in range(0, npair, OHG):
                            gn = min(OHG, npair - j0)
                            pc0 = call.chunks[j0][1]
                            ohg = ohpool.tile([128, OHG * 128], bf, tag="oh", name=f"oh_{L}_{pc0}")
                            nc.vector.tensor_tensor(
                                out=ohg[:, :gn * 128].rearrange("p (a f) -> p a f", a=gn),
                                in0=iota.rearrange("p (a f) -> p a f", a=1).to_broadcast([128, gn, 128]),
                                in1=meta_sb[:, pc0:pc0 + gn].to_broadcast([128, gn, 128]),
                                op=mybir.AluOpType.is_equal,
                            )
                            for jj in range(gn):
                                t, pc, first, last, jc = call.chunks[j0 + jj]
                                nc.tensor.matmul(
                                    out=aggs[t][:],
                                    lhsT=ohg[:, jj * 128:(jj + 1) * 128],
                                    rhs=g[:, jc: jc + 1, :],
                                    start=first,
                                    stop=last,
                                )

                    for t in blk:
                        tsl = slice(t * 128, (t + 1) * 128)
                        magg = hpool.tile([128, 128], bf, tag="magg", name=f"magg_{L}_{t}")
                        nc.scalar.activation(
                            out=magg[:], in_=aggs[t][:],
                            func=mybir.ActivationFunctionType.Identity,
                            bias=0.0, scale=fb_sb[:, 2 + t: 3 + t],
                        )
                        hp = mm_psum.tile([128, 128], f32, tag="mm", name=f"hp_{L}_{t}")
                        nc.tensor.matmul(out=hp[:], lhsT=wr, rhs=srcT[:, tsl],
                                         start=True, stop=False)
                        nc.tensor.matmul(out=hp[:], lhsT=b_row, rhs=ones_row,
                                         start=False, stop=False)
                        nc.tensor.matmul(out=hp[:], lhsT=magg[:], rhs=ident,
                                         start=False, stop=True)
                        if L == 0:
                            nc.scalar.activation(
                                out=h1t_sb[:, tsl], in_=hp[:],
                                func=mybir.ActivationFunctionType.Relu,
                            )
                            w2l = wbf_sb[:, W2L:W2L + 128]
                            yp1 = mm_psum.tile([128, 128], f32, tag="mm", name=f"yp1_{t}")
                            nc.tensor.matmul(
                                out=yp1[:], lhsT=h1t_sb[:, tsl],
                                rhs=w2l, start=True, stop=True,
                            )
                            yl1_sb = hpool.tile([128, 128], bf, tag="ylsb", name=f"yl1sb_{t}")
                            nc.scalar.copy(out=yl1_sb[:], in_=yp1[:])
                            nc.sync.dma_start(out=y_local[tsl, :], in_=yl1_sb[:])
                        else:
                            hT = hpool.tile([128, 128], bf, tag="hT", name=f"hT_{t}")
                            rs = hpool.tile([128, 1], f32, tag="rs", name=f"rs_{t}")
                            nc.scalar.activation(
                                out=hT[:], in_=hp[:],
                                func=mybir.ActivationFunctionType.Relu,
                                accum_out=rs[:],
                            )
                            ap_ = mm_psum.tile([1, 128], f32, tag="mm", name=f"act_{t}")
                            nc.tensor.matmul(
                                out=ap_[:], lhsT=wbf_sb[:, WACOL:WACOL + 1], rhs=hT[:],
                                start=True, stop=True,
                            )
                            arow = hpool.tile([1, 128], f32, tag="arow", name=f"arow_{t}")
                            nc.scalar.activation(
                                out=arow[:], in_=ap_[:],
                                func=mybir.ActivationFunctionType.Identity,
                                bias=fb_sb[0:1, 0:1], scale=1.0,
                            )
                            nc.sync.dma_start(
                                out=actor_ext[0:1, tsl], in_=arow[:]
                            )
                            nc.vector.tensor_add(out=crit_acc[:], in0=crit_acc[:], in1=rs[:])

            cp = mm_psum.tile([1, 1], f32, tag="mm", name="critp")
            nc.tensor.matmul(out=cp[:], lhsT=crit_acc[:], rhs=fb_sb[:, 1:2], start=True, stop=True)
            crit_sb = hpool.tile([1, 1], f32, tag="csb", name="crit_sb")
            nc.scalar.copy(out=crit_sb[:], in_=cp[:])
            nc.sync.dma_start(out=crit_ext[:], in_=crit_sb[:])

    nc.finalize()
    return nc


def kernel(x, edge_index, W1_l, b1, W1_r, W2_l, b2, W2_r, Wa, ba, Wc, bc):
    global LAST_RESULT
    x = np.asarray(x)
    assert x.shape == (N, D)

    sched, planes = _build_schedule(np.asarray(edge_index))
    nc = _build_kernel(sched)

    xpad = np.zeros((NPAD, D), dtype=np.float32)
    xpad[:N] = np.asarray(x, np.float32)

    wbf = np.zeros((128, WBFW), dtype=BF16)
    wbf[:, W1L:W1L + 128] = np.asarray(W1_l, np.float32).T.astype(BF16)
    wbf[:, W1R:W1R + 128] = np.asarray(W1_r, np.float32).T.astype(BF16)
    wbf[:, W2L:W2L + 128] = np.asarray(W2_l, np.float32).T.astype(BF16)
    wbf[:, W2R:W2R + 128] = np.asarray(W2_r, np.float32).T.astype(BF16)
    wbf[:, WACOL] = np.asarray(Wa, np.float32)[0].astype(BF16)
    wbf[:, IOTA:IOTA + 128] = np.tile(
        np.arange(128, dtype=np.float32)[None, :], (128, 1)).astype(BF16)
    wbf[:, IDENT:IDENT + 128] = np.eye(128, dtype=np.float32).astype(BF16)
    wbf[0, ONESROW:ONESROW + 128] = np.ones(128, np.float32).astype(BF16)
    wbf[0, B1ROW:B1ROW + 128] = np.asarray(b1, np.float32).astype(BF16)
    wbf[0, B2ROW:B2ROW + 128] = np.asarray(b2, np.float32).astype(BF16)

    in_maps = []
    for p in range(NCORES):
        idx_plane, meta, inv_p = planes[p]
        fb = np.zeros((128, FBW), dtype=np.float32)
        fb[0, 0] = np.float32(np.asarray(ba).reshape(-1)[0])
        fb[:, 1] = np.asarray(Wc, np.float32)[0]
        fb[:, 2:] = inv_p
        xt = np.ascontiguousarray(xpad[p * VPC:(p + 1) * VPC].T).astype(BF16)
        in_maps.append({
            "xt": xt, "gidx": idx_plane, "meta": meta, "wbf": wbf, "fb": fb,
        })

    res = run_bass_kernel_spmd(nc, in_maps, core_ids=list(range(NCORES)))
    LAST_RESULT = res

    actor = np.concatenate([res.results[p]["actor"][0] for p in range(NCORES)])[:N]
    crit_sum = np.sum([res.results[p]["crit"][0, 0] for p in range(NCORES)])
    critic = np.float32(crit_sum / N + np.float32(np.asarray(bc).reshape(-1)[0]))
    return actor.astype(np.float32), critic


# revision 24
# speedup vs baseline: 1.0699x; 1.0699x over previous
"""Distributed SAGE GNN kernel for 8 TRN2 NeuronCores.

Strategy (per sharding hint): nodes and their output rows are sharded across
the 8 cores; edges are partitioned by destination core. Weights replicated.

Per layer:
  1. pre-transform: y = input @ W_l.T computed on each core's node shard
     (feature-major input tiles as matmul lhsT), then one AllGather so every
     core holds the full y (message table) in its HBM.
  2. aggregation: for each 128-node destination tile, gather y[src] rows with
     dma_gather (bf16 rows, int16 window-local indices, 4 source windows of
     25088 rows so indices fit int16; 4 SWDGE queues round-robin) and
     accumulate  agg[n, f] = sum_e onehot[e, n] * y[src_e, f]  as one-hot x
     message matmuls in PSUM. One-hots are built 16 pairs per DVE op via
     broadcast is_equal against an iota tile; 128-edge chunks may straddle
     tile boundaries (one matmul per (chunk, tile) pair, the dst-relative
     meta column masks foreign edges).
  3. finalize: h^T = relu(W_r.T-term + bias + agg/deg) fused into one PSUM
     accumulation group (bias via a K=1 matmul, agg via a transposing matmul
     with lhsT=agg), the 1/deg mean scale applied by the Scalar engine on the
     PSUM->SBUF copy. Layer 2's pre-transform is fused into layer 1's
     finalize; actor/critic heads are tiny matmuls/reductions in the L2 loop.
"""

import numpy as np
import ml_dtypes

import concourse.bass as bass
import concourse.mybir as mybir
import concourse.tile as tile
from concourse import bacc
from concourse.bass_utils import run_bass_kernel_spmd

BF16 = ml_dtypes.bfloat16

N = 100000
E = 1600000
D = 128
NCORES = 8
VPC = 12544            # nodes per core (padded)
NPAD = VPC * NCORES    # 100352
TPC = VPC // 128       # 98 dst tiles per core
NSRC = 4               # gather source windows (int16 index limit)
SRCW = NPAD // NSRC    # 25088
TB = 3                 # dst tiles per block (PSUM-resident accumulators)
MAXCH = 32             # max 128-edge chunks per dma_gather call
OHG = 16               # pairs per batched one-hot DVE op

# wbf plane column layout (bf16)
W1L, W1R, W2L, W2R = 0, 128, 256, 384
WACOL = 512
IOTA, IDENT = 514, 642
ONESROW, B1ROW, B2ROW = 770, 898, 1026
WBFW = 1154
# fb plane (f32): [0,0]=ba, col1 = Wc, cols 2.. = 1/deg per tile
FBW = 2 + TPC

LAST_RESULT = None     # BassKernelResults of the most recent run (for tests)


class _Call:
    __slots__ = ("s", "nch", "icol", "chunks", "tailpad")

    def __init__(self, s, icol):
        self.s = s
        self.nch = 0
        self.icol = icol      # column offset into the idx plane
        self.chunks = []      # list of (t, pair_col, first, last, chunk_in_call)
        self.tailpad = 0      # trailing pad slots (gather row 0, masked)


def _build_schedule(edge_index):
    """Host-side edge partitioning. Returns (sched, per-core planes).

    Edge stream layout per core (identical shape on every core): for each
    block of TB dst tiles, for each source window s, the edges of groups
    (s, t in blk) are packed back-to-back, each group padded to the max count
    over cores (pads: idx 0, sentinel dst). The packed segment is rounded up
    to a multiple of 128. 128-edge chunks may straddle tile boundaries; each
    (chunk, tile) pair becomes one one-hot matmul whose dst-relative meta
    column masks out the other tiles' edges.
    """
    src = np.asarray(edge_index[0], dtype=np.int64)
    dst = np.asarray(edge_index[1], dtype=np.int64)

    core = dst // VPC
    t_loc = (dst - core * VPC) // 128
    s_chunk = src // SRCW
    grp = (core * NSRC + s_chunk) * TPC + t_loc          # [E]

    cnt = np.bincount(grp, minlength=NCORES * NSRC * TPC)
    cnt = cnt.reshape(NCORES, NSRC, TPC)
    C = cnt.max(axis=0)                                  # exact per (s, t)

    deg = np.bincount(dst, minlength=NPAD).astype(np.float64)
    invdeg = (1.0 / np.maximum(deg, 1.0)).astype(np.float32)   # [NPAD]

    blocks = [list(range(b, min(b + TB, TPC))) for b in range(0, TPC, TB)]

    slot_base = np.zeros((NSRC, TPC), dtype=np.int64)
    seg_meta = []            # per (blk, s): (slot0, seg_len, pad_len, groups)
    off = 0
    for blk in blocks:
        for s in range(NSRC):
            g0 = off
            groups = []
            for t in blk:
                slot_base[s, t] = off
                groups.append((t, off - g0, int(C[s, t])))
                off += int(C[s, t])
            seg_len = off - g0
            pad = (-seg_len) % 128
            off += pad
            seg_meta.append((g0, seg_len, pad, groups))
    etot = off
    assert etot % 128 == 0

    # total (chunk, tile) pairs per tile, for start/stop flags
    t_total = np.zeros(TPC, dtype=np.int64)
    for (g0, seg_len, pad, groups) in seg_meta:
        nch_seg = (seg_len + pad) // 128
        for k in range(nch_seg):
            a, b = k * 128, k * 128 + 128
            for (t, goff, glen) in groups:
                if glen and goff < b and a < goff + glen:
                    t_total[t] += 1

    calls_by_blk = [[] for _ in blocks]
    t_seen = np.zeros(TPC, dtype=np.int64)
    pc = 0
    icol = 0
    seg_i = 0
    pair_info = []               # (slot0_of_chunk, t) per pair
    for bi, blk in enumerate(blocks):
        calls = calls_by_blk[bi]
        for s in range(NSRC):
            g0, seg_len, pad, groups = seg_meta[seg_i]
            seg_i += 1
            nch_seg = (seg_len + pad) // 128
            call = None
            for k in range(nch_seg):
                if call is None or call.nch == MAXCH:
                    if call is not None:
                        calls.append(call)
                    call = _Call(s, icol)
                a, b = k * 128, k * 128 + 128
                j_in_call = call.nch
                for (t, goff, glen) in groups:
                    if glen and goff < b and a < goff + glen:
                        first = t_seen[t] == 0
                        t_seen[t] += 1
                        last = t_seen[t] == t_total[t]
                        call.chunks.append((t, pc, bool(first), bool(last), j_in_call))
                        pair_info.append((g0 + a, t))
                        pc += 1
                call.nch += 1
                icol += 8
            call.tailpad = pad
            calls.append(call)
            call = None
    n_pairs = pc
    idx_cols = icol

    # ---- per-core slot arrays -------------------------------------------
    order = np.argsort(grp, kind="stable")
    sg = grp[order]
    n_groups = NCORES * NSRC * TPC
    gstart = np.searchsorted(sg, np.arange(n_groups))
    rank = np.arange(E, dtype=np.int64) - gstart[sg]
    s_o = s_chunk[order]
    t_o = t_loc[order]
    core_o = core[order]
    src_o = src[order]
    dst_o = dst[order]
    slotpos = slot_base[s_o, t_o] + rank

    planes = []
    for p in range(NCORES):
        m = core_o == p
        sl = slotpos[m]
        s_src = np.zeros(etot, dtype=np.int16)
        s_dst = np.full(etot, 999.0, dtype=np.float32)
        s_tile = np.full(etot, -1, dtype=np.int64)
        s_src[sl] = (src_o[m] - s_o[m] * SRCW).astype(np.int16)
        s_dst[sl] = (dst_o[m] - (p * VPC + t_o[m] * 128)).astype(np.float32)
        s_tile[sl] = t_o[m]

        idx_plane = np.zeros((16, idx_cols), dtype=np.int16)
        meta = np.full((128, n_pairs), 999.0, dtype=np.float32)
        for calls in calls_by_blk:
            for call in calls:
                nidx = call.nch * 128
                base_slot = None
                for (t, pcx, first, last, j_in_call) in call.chunks:
                    if base_slot is None:
                        base_slot = pair_info[pcx][0] - j_in_call * 128
                    a = pair_info[pcx][0]
                    col = s_dst[a:a + 128].copy()
                    col[s_tile[a:a + 128] != t] = 999.0
                    meta[:, pcx] = col
                arr = s_src[base_slot: base_slot + nidx]
                idx_plane[:, call.icol: call.icol + nidx // 16] = (
                    arr.reshape(-1, 16).T
                )
        idx_plane = np.tile(idx_plane, (8, 1))

        inv_p = invdeg[p * VPC:(p + 1) * VPC].reshape(TPC, 128).T  # [128, TPC]
        planes.append((idx_plane, meta.astype(BF16), np.ascontiguousarray(inv_p)))

    sched = {
        "t_zero": {int(t) for t in range(TPC) if t_total[t] == 0},
        "blocks": blocks,
        "calls_by_blk": calls_by_blk,
        "n_chunks": n_pairs,
        "idx_cols": idx_cols,
        "etot": etot,
        "max_nch": max(c.nch for calls in calls_by_blk for c in calls),
    }
    return sched, planes


def _build_kernel(sched):
    nc = bacc.Bacc(num_devices=NCORES, num_swdge_queues=4)
    bf = mybir.dt.bfloat16
    f32 = mybir.dt.float32
    n_chunks = sched["n_chunks"]
    idx_cols = sched["idx_cols"]
    max_nch = sched["max_nch"]

    xt_ext = nc.declare_dram_parameter("xt", [D, VPC], bf, isOutput=False)
    gidx_ext = nc.declare_dram_parameter("gidx", [128, idx_cols], mybir.dt.int16, isOutput=False)
    meta_ext = nc.declare_dram_parameter("meta", [128, n_chunks], bf, isOutput=False)
    wbf_ext = nc.declare_dram_parameter("wbf", [128, WBFW], bf, isOutput=False)
    fb_ext = nc.declare_dram_parameter("fb", [128, FBW], f32, isOutput=False)
    actor_ext = nc.declare_dram_parameter("actor", [1, VPC], f32, isOutput=True)
    crit_ext = nc.declare_dram_parameter("crit", [1, 1], f32, isOutput=True)

    with tile.TileContext(nc) as tc:
        with (
            tc.tile_pool(name="cpool", bufs=1) as cpool,
            tc.tile_pool(name="gpool", bufs=12) as gpool,
            tc.tile_pool(name="ohpool", bufs=12) as ohpool,
            tc.tile_pool(name="hpool", bufs=4) as hpool,
            tc.tile_pool(name="agg_psum", bufs=6, space="PSUM") as agg_psum,
            tc.tile_pool(name="mm_psum", bufs=2, space="PSUM") as mm_psum,
            tc.tile_pool(name="dram", bufs=1, space="DRAM") as dram,
        ):
            gidx_sb = cpool.tile([128, idx_cols], mybir.dt.int16)
            nc.sync.dma_start(out=gidx_sb[:], in_=gidx_ext[:])
            meta_sb = cpool.tile([128, n_chunks], bf)
            nc.sync.dma_start(out=meta_sb[:], in_=meta_ext[:])
            wbf_sb = cpool.tile([128, WBFW], bf)
            nc.sync.dma_start(out=wbf_sb[:], in_=wbf_ext[:])
            fb_sb = cpool.tile([128, FBW], f32)
            nc.sync.dma_start(out=fb_sb[:], in_=fb_ext[:])
            xt_sb = cpool.tile([128, VPC], bf)
            nc.sync.dma_start(out=xt_sb[:], in_=xt_ext[:])
            h1t_sb = cpool.tile([128, VPC], bf)

            iota = wbf_sb[:, IOTA:IOTA + 128]
            ident = wbf_sb[:, IDENT:IDENT + 128]
            ones_row = wbf_sb[0:1, ONESROW:ONESROW + 128]

            crit_acc = cpool.tile([128, 1], f32)
            nc.vector.memset(crit_acc[:], 0.0)

            y_local = dram.tile([VPC, D], bf)
            y_full0 = dram.tile([NPAD, D], bf, addr_space="Shared")
            y_full1 = dram.tile([NPAD, D], bf, addr_space="Shared")

            qctr = [0]
            for L in range(2):
                y_full = y_full0 if L == 0 else y_full1
                srcT = xt_sb if L == 0 else h1t_sb
                wl = wbf_sb[:, (W1L if L == 0 else W2L):(W1L if L == 0 else W2L) + 128]
                wr = wbf_sb[:, (W1R if L == 0 else W2R):(W1R if L == 0 else W2R) + 128]
                b_row = wbf_sb[0:1, (B1ROW if L == 0 else B2ROW):(B1ROW if L == 0 else B2ROW) + 128]

                # ---- pre-transform (L1's is fused into L0's finalize below)
                if L == 0:
                    for t in range(TPC):
                        yp = mm_psum.tile([128, 128], f32, tag="mm", name=f"yp_{L}_{t}")
                        nc.tensor.matmul(
                            out=yp[:], lhsT=srcT[:, t * 128:(t + 1) * 128], rhs=wl,
                            start=True, stop=True,
                        )
                        yl_sb = hpool.tile([128, 128], bf, tag="ylsb", name=f"ylsb_{L}_{t}")
                        nc.scalar.copy(out=yl_sb[:], in_=yp[:])
                        nc.sync.dma_start(out=y_local[t * 128:(t + 1) * 128, :], in_=yl_sb[:])

                nc.gpsimd.collective_compute(
                    "AllGather",
                    mybir.AluOpType.bypass,
                    replica_groups=[list(range(NCORES))],
                    ins=[y_local.opt()],
                    outs=[y_full.opt()],
                )

                # ---- aggregation + finalize, per destination-tile block
                for bi, blk in enumerate(sched["blocks"]):
                    aggs = {}
                    for t in blk:
                        aggs[t] = agg_psum.tile([128, 128], f32, tag="agg", name=f"agg_{L}_{t}")
                    for ci, call in enumerate(sched["calls_by_blk"][bi]):
                        g = gpool.tile([128, max_nch, 128], bf, tag="g", name=f"g_{L}_{bi}_{call.icol}")
                        nidx = call.nch * 128
                        nc.gpsimd.dma_gather(
                            out_ap=g[:, :call.nch, :],
                            in_ap=y_full[call.s * SRCW:(call.s + 1) * SRCW, :],
                            idxs_ap=gidx_sb[:, call.icol: call.icol + nidx // 16],
                            num_idxs=nidx,
                            num_idxs_reg=nidx,
                            elem_size=D,
                            single_packet=False,
                            queue_num=qctr[0] % 4,
                        )
                        qctr[0] += 1
                        npair = len(call.chunks)
                        for j0 in range(0, npair, OHG):
                            gn = min(OHG, npair - j0)
                            pc0 = call.chunks[j0][1]
                            ohg = ohpool.tile([128, OHG * 128], bf, tag="oh", name=f"oh_{L}_{pc0}")
                            nc.vector.tensor_tensor(
                                out=ohg[:, :gn * 128].rearrange("p (a f) -> p a f", a=gn),
                                in0=iota.rearrange("p (a f) -> p a f", a=1).to_broadcast([128, gn, 128]),
                                in1=meta_sb[:, pc0:pc0 + gn].to_broadcast([128, gn, 128]),
                                op=mybir.AluOpType.is_equal,
                            )
                            for jj in range(gn):
                                t, pc, first, last, jc = call.chunks[j0 + jj]
                                nc.tensor.matmul(
                                    out=aggs[t][:],
                                    lhsT=ohg[:, jj * 128:(jj + 1) * 128],
                                    rhs=g[:, jc: jc + 1, :],
                                    start=first,
                                    stop=last,
                                )

                    for t in blk:
                        tsl = slice(t * 128, (t + 1) * 128)
                        magg = hpool.tile([128, 128], bf, tag="magg", name=f"magg_{L}_{t}")
                        if t in sched["t_zero"]:
                            nc.vector.memset(magg[:], 0.0)
                        else:
                            nc.scalar.activation(
                                out=magg[:], in_=aggs[t][:],
                                func=mybir.ActivationFunctionType.Identity,
                                bias=0.0, scale=fb_sb[:, 2 + t: 3 + t],
                            )
                        hp = mm_psum.tile([128, 128], f32, tag="mm", name=f"hp_{L}_{t}")
                        nc.tensor.matmul(out=hp[:], lhsT=wr, rhs=srcT[:, tsl],
                                         start=True, stop=False)
                        nc.tensor.matmul(out=hp[:], lhsT=b_row, rhs=ones_row,
                                         start=False, stop=False)
                        nc.tensor.matmul(out=hp[:], lhsT=magg[:], rhs=ident,
                                         start=False, stop=True)
                        if L == 0:
                            nc.scalar.activation(
                                out=h1t_sb[:, tsl], in_=hp[:],
                                func=mybir.ActivationFunctionType.Relu,
                            )
                            w2l = wbf_sb[:, W2L:W2L + 128]
                            yp1 = mm_psum.tile([128, 128], f32, tag="mm", name=f"yp1_{t}")
                            nc.tensor.matmul(
                                out=yp1[:], lhsT=h1t_sb[:, tsl],
                                rhs=w2l, start=True, stop=True,
                            )
                            yl1_sb = hpool.tile([128, 128], bf, tag="ylsb", name=f"yl1sb_{t}")
                            nc.scalar.copy(out=yl1_sb[:], in_=yp1[:])
                            nc.sync.dma_start(out=y_local[tsl, :], in_=yl1_sb[:])
                        else:
                            hT = hpool.tile([128, 128], bf, tag="hT", name=f"hT_{t}")
                            rs = hpool.tile([128, 1], f32, tag="rs", name=f"rs_{t}")
                            nc.scalar.activation(
                                out=hT[:], in_=hp[:],
                                func=mybir.ActivationFunctionType.Relu,
                                accum_out=rs[:],
                            )
                            ap_ = mm_psum.tile([1, 128], f32, tag="mm", name=f"act_{t}")
                            nc.tensor.matmul(
                                out=ap_[:], lhsT=wbf_sb[:, WACOL:WACOL + 1], rhs=hT[:],
                                start=True, stop=True,
                            )
                            arow = hpool.tile([1, 128], f32, tag="arow", name=f"arow_{t}")
                            nc.scalar.activation(
                                out=arow[:], in_=ap_[:],
                                func=mybir.ActivationFunctionType.Identity,
                                bias=fb_sb[0:1, 0:1], scale=1.0,
                            )
                            nc.sync.dma_start(
                                out=actor_ext[0:1, tsl], in_=arow[:]
                            )
                            nc.vector.tensor_add(out=crit_acc[:], in0=crit_acc[:], in1=rs[:])

            cp = mm_psum.tile([1, 1], f32, tag="mm", name="critp")
            nc.tensor.matmul(out=cp[:], lhsT=crit_acc[:], rhs=fb_sb[:, 1:2], start=True, stop=True)
            crit_sb = hpool.tile([1, 1], f32, tag="csb", name="crit_sb")
            nc.scalar.copy(out=crit_sb[:], in_=cp[:])
            nc.sync.dma_start(out=crit_ext[:], in_=crit_sb[:])

    nc.finalize()
    return nc


def kernel(x, edge_index, W1_l, b1, W1_r, W2_l, b2, W2_r, Wa, ba, Wc, bc):
    global LAST_RESULT
    x = np.asarray(x)
    assert x.shape == (N, D)

    sched, planes = _build_schedule(np.asarray(edge_index))
    nc = _build_kernel(sched)

    xpad = np.zeros((NPAD, D), dtype=np.float32)
    xpad[:N] = np.asarray(x, np.float32)

    wbf = np.zeros((128, WBFW), dtype=BF16)
    wbf[:, W1L:W1L + 128] = np.asarray(W1_l, np.float32).T.astype(BF16)
    wbf[:, W1R:W1R + 128] = np.asarray(W1_r, np.float32).T.astype(BF16)
    wbf[:, W2L:W2L + 128] = np.asarray(W2_l, np.float32).T.astype(BF16)
    wbf[:, W2R:W2R + 128] = np.asarray(W2_r, np.float32).T.astype(BF16)
    wbf[:, WACOL] = np.asarray(Wa, np.float32)[0].astype(BF16)
    wbf[:, IOTA:IOTA + 128] = np.tile(
        np.arange(128, dtype=np.float32)[None, :], (128, 1)).astype(BF16)
    wbf[:, IDENT:IDENT + 128] = np.eye(128, dtype=np.float32).astype(BF16)
    wbf[0, ONESROW:ONESROW + 128] = np.ones(128, np.float32).astype(BF16)
    wbf[0, B1ROW:B1ROW + 128] = np.asarray(b1, np.float32).astype(BF16)
    wbf[0, B2ROW:B2ROW + 128] = np.asarray(b2, np.float32).astype(BF16)

    in_maps = []
    for p in range(NCORES):
        idx_plane, meta, inv_p = planes[p]
        fb = np.zeros((128, FBW), dtype=np.float32)
        fb[0, 0] = np.float32(np.asarray(ba).reshape(-1)[0])
        fb[:, 1] = np.asarray(Wc, np.float32)[0]
        fb[:, 2:] = inv_p
        xt = np.ascontiguousarray(xpad[p * VPC:(p + 1) * VPC].T).astype(BF16)
        in_maps.append({
            "xt": xt, "gidx": idx_plane, "meta": meta, "wbf": wbf, "fb": fb,
        })

    res = run_bass_kernel_spmd(nc, in_maps, core_ids=list(range(NCORES)))
    LAST_RESULT = res

    actor = np.concatenate([res.results[p]["actor"][0] for p in range(NCORES)])[:N]
    crit_sum = np.sum([res.results[p]["crit"][0, 0] for p in range(NCORES)])
    critic = np.float32(crit_sum / N + np.float32(np.asarray(bc).reshape(-1)[0]))
    return actor.astype(np.float32), critic


# revision 25
# speedup vs baseline: 1.1582x; 1.0825x over previous
"""Distributed SAGE GNN kernel for 8 TRN2 NeuronCores.

Strategy (per sharding hint): nodes and their output rows are sharded across
the 8 cores; edges are partitioned by destination core. Weights replicated.

Per layer:
  1. pre-transform: y = input @ W_l.T computed on each core's node shard
     (feature-major input tiles as matmul lhsT), then one AllGather so every
     core holds the full y (message table) in its HBM.
  2. aggregation: for each 128-node destination tile, gather y[src] rows with
     dma_gather (bf16 rows, int16 window-local indices, 4 source windows of
     25088 rows so indices fit int16; 4 SWDGE queues round-robin) and
     accumulate  agg[n, f] = sum_e onehot[e, n] * y[src_e, f]  as one-hot x
     message matmuls in PSUM. One-hots are built 16 pairs per DVE op via
     broadcast is_equal against an iota tile; 128-edge chunks may straddle
     tile boundaries (one matmul per (chunk, tile) pair, the dst-relative
     meta column masks foreign edges).
  3. finalize: h^T = relu(W_r.T-term + bias + agg/deg) fused into one PSUM
     accumulation group (bias via a K=1 matmul, agg via a transposing matmul
     with lhsT=agg), the 1/deg mean scale applied by the Scalar engine on the
     PSUM->SBUF copy. Layer 2's pre-transform is fused into layer 1's
     finalize; actor/critic heads are tiny matmuls/reductions in the L2 loop.
"""

import numpy as np
import ml_dtypes

import concourse.bass as bass
import concourse.mybir as mybir
import concourse.tile as tile
from concourse import bacc
from concourse.bass_utils import run_bass_kernel_spmd

BF16 = ml_dtypes.bfloat16

N = 100000
E = 1600000
D = 128
NCORES = 8
VPC = 12544            # nodes per core (padded)
NPAD = VPC * NCORES    # 100352
TPC = VPC // 128       # 98 dst tiles per core
NSRC = 4               # gather source windows (int16 index limit)
SRCW = NPAD // NSRC    # 25088
TB = 3                 # dst tiles per block (PSUM-resident accumulators)
MAXCH = 32             # max 128-edge chunks per dma_gather call
OHG = 32               # pairs per batched one-hot DVE op

# wbf plane column layout (bf16)
W1L, W1R, W2L, W2R = 0, 128, 256, 384
WACOL = 512
IOTA, IDENT = 514, 642
ONESROW, B1ROW, B2ROW = 770, 898, 1026
WBFW = 1154
# fb plane (f32): [0,0]=ba, col1 = Wc, cols 2.. = 1/deg per tile
FBW = 2 + TPC

LAST_RESULT = None     # BassKernelResults of the most recent run (for tests)


class _Call:
    __slots__ = ("s", "nch", "icol", "chunks", "tailpad")

    def __init__(self, s, icol):
        self.s = s
        self.nch = 0
        self.icol = icol      # column offset into the idx plane
        self.chunks = []      # list of (t, pair_col, first, last, chunk_in_call)
        self.tailpad = 0      # trailing pad slots (gather row 0, masked)


def _build_schedule(edge_index):
    """Host-side edge partitioning. Returns (sched, per-core planes).

    Edge stream layout per core (identical shape on every core): for each
    block of TB dst tiles, for each source window s, the edges of groups
    (s, t in blk) are packed back-to-back, each group padded to the max count
    over cores (pads: idx 0, sentinel dst). The packed segment is rounded up
    to a multiple of 128. 128-edge chunks may straddle tile boundaries; each
    (chunk, tile) pair becomes one one-hot matmul whose dst-relative meta
    column masks out the other tiles' edges.
    """
    src = np.asarray(edge_index[0], dtype=np.int64)
    dst = np.asarray(edge_index[1], dtype=np.int64)

    core = dst // VPC
    t_loc = (dst - core * VPC) // 128
    s_chunk = src // SRCW
    grp = (core * NSRC + s_chunk) * TPC + t_loc          # [E]

    cnt = np.bincount(grp, minlength=NCORES * NSRC * TPC)
    cnt = cnt.reshape(NCORES, NSRC, TPC)
    C = cnt.max(axis=0)                                  # exact per (s, t)

    deg = np.bincount(dst, minlength=NPAD).astype(np.float64)
    invdeg = (1.0 / np.maximum(deg, 1.0)).astype(np.float32)   # [NPAD]

    blocks = [list(range(b, min(b + TB, TPC))) for b in range(0, TPC, TB)]

    slot_base = np.zeros((NSRC, TPC), dtype=np.int64)
    seg_meta = []            # per (blk, s): (slot0, seg_len, pad_len, groups)
    off = 0
    for blk in blocks:
        for s in range(NSRC):
            g0 = off
            groups = []
            for t in blk:
                slot_base[s, t] = off
                groups.append((t, off - g0, int(C[s, t])))
                off += int(C[s, t])
            seg_len = off - g0
            pad = (-seg_len) % 128
            off += pad
            seg_meta.append((g0, seg_len, pad, groups))
    etot = off
    assert etot % 128 == 0

    # total (chunk, tile) pairs per tile, for start/stop flags
    t_total = np.zeros(TPC, dtype=np.int64)
    for (g0, seg_len, pad, groups) in seg_meta:
        nch_seg = (seg_len + pad) // 128
        for k in range(nch_seg):
            a, b = k * 128, k * 128 + 128
            for (t, goff, glen) in groups:
                if glen and goff < b and a < goff + glen:
                    t_total[t] += 1

    calls_by_blk = [[] for _ in blocks]
    t_seen = np.zeros(TPC, dtype=np.int64)
    pc = 0
    icol = 0
    seg_i = 0
    pair_info = []               # (slot0_of_chunk, t) per pair
    for bi, blk in enumerate(blocks):
        calls = calls_by_blk[bi]
        for s in range(NSRC):
            g0, seg_len, pad, groups = seg_meta[seg_i]
            seg_i += 1
            nch_seg = (seg_len + pad) // 128
            call = None
            for k in range(nch_seg):
                if call is None or call.nch == MAXCH:
                    if call is not None:
                        calls.append(call)
                    call = _Call(s, icol)
                a, b = k * 128, k * 128 + 128
                j_in_call = call.nch
                for (t, goff, glen) in groups:
                    if glen and goff < b and a < goff + glen:
                        first = t_seen[t] == 0
                        t_seen[t] += 1
                        last = t_seen[t] == t_total[t]
                        call.chunks.append((t, pc, bool(first), bool(last), j_in_call))
                        pair_info.append((g0 + a, t))
                        pc += 1
                call.nch += 1
                icol += 8
            call.tailpad = pad
            calls.append(call)
            call = None
    n_pairs = pc
    idx_cols = icol

    # ---- per-core slot arrays -------------------------------------------
    order = np.argsort(grp, kind="stable")
    sg = grp[order]
    n_groups = NCORES * NSRC * TPC
    gstart = np.searchsorted(sg, np.arange(n_groups))
    rank = np.arange(E, dtype=np.int64) - gstart[sg]
    s_o = s_chunk[order]
    t_o = t_loc[order]
    core_o = core[order]
    src_o = src[order]
    dst_o = dst[order]
    slotpos = slot_base[s_o, t_o] + rank

    planes = []
    for p in range(NCORES):
        m = core_o == p
        sl = slotpos[m]
        s_src = np.zeros(etot, dtype=np.int16)
        s_dst = np.full(etot, 999.0, dtype=np.float32)
        s_tile = np.full(etot, -1, dtype=np.int64)
        s_src[sl] = (src_o[m] - s_o[m] * SRCW).astype(np.int16)
        s_dst[sl] = (dst_o[m] - (p * VPC + t_o[m] * 128)).astype(np.float32)
        s_tile[sl] = t_o[m]

        idx_plane = np.zeros((16, idx_cols), dtype=np.int16)
        meta = np.full((128, n_pairs), 999.0, dtype=np.float32)
        for calls in calls_by_blk:
            for call in calls:
                nidx = call.nch * 128
                base_slot = None
                for (t, pcx, first, last, j_in_call) in call.chunks:
                    if base_slot is None:
                        base_slot = pair_info[pcx][0] - j_in_call * 128
                    a = pair_info[pcx][0]
                    col = s_dst[a:a + 128].copy()
                    col[s_tile[a:a + 128] != t] = 999.0
                    meta[:, pcx] = col
                arr = s_src[base_slot: base_slot + nidx]
                idx_plane[:, call.icol: call.icol + nidx // 16] = (
                    arr.reshape(-1, 16).T
                )
        idx_plane = np.tile(idx_plane, (8, 1))

        inv_p = invdeg[p * VPC:(p + 1) * VPC].reshape(TPC, 128).T  # [128, TPC]
        planes.append((idx_plane, meta.astype(BF16), np.ascontiguousarray(inv_p)))

    sched = {
        "t_zero": {int(t) for t in range(TPC) if t_total[t] == 0},
        "blocks": blocks,
        "calls_by_blk": calls_by_blk,
        "n_chunks": n_pairs,
        "idx_cols": idx_cols,
        "etot": etot,
        "max_nch": max(c.nch for calls in calls_by_blk for c in calls),
    }
    return sched, planes


def _build_kernel(sched):
    nc = bacc.Bacc(num_devices=NCORES, num_swdge_queues=4)
    bf = mybir.dt.bfloat16
    f32 = mybir.dt.float32
    n_chunks = sched["n_chunks"]
    idx_cols = sched["idx_cols"]
    max_nch = sched["max_nch"]

    xt_ext = nc.declare_dram_parameter("xt", [D, VPC], bf, isOutput=False)
    gidx_ext = nc.declare_dram_parameter("gidx", [128, idx_cols], mybir.dt.int16, isOutput=False)
    meta_ext = nc.declare_dram_parameter("meta", [128, n_chunks], bf, isOutput=False)
    wbf_ext = nc.declare_dram_parameter("wbf", [128, WBFW], bf, isOutput=False)
    fb_ext = nc.declare_dram_parameter("fb", [128, FBW], f32, isOutput=False)
    actor_ext = nc.declare_dram_parameter("actor", [1, VPC], f32, isOutput=True)
    crit_ext = nc.declare_dram_parameter("crit", [1, 1], f32, isOutput=True)

    with tile.TileContext(nc) as tc:
        with (
            tc.tile_pool(name="cpool", bufs=1) as cpool,
            tc.tile_pool(name="gpool", bufs=12) as gpool,
            tc.tile_pool(name="ohpool", bufs=6) as ohpool,
            tc.tile_pool(name="hpool", bufs=4) as hpool,
            tc.tile_pool(name="agg_psum", bufs=6, space="PSUM") as agg_psum,
            tc.tile_pool(name="mm_psum", bufs=2, space="PSUM") as mm_psum,
            tc.tile_pool(name="dram", bufs=1, space="DRAM") as dram,
        ):
            gidx_sb = cpool.tile([128, idx_cols], mybir.dt.int16)
            nc.sync.dma_start(out=gidx_sb[:], in_=gidx_ext[:])
            meta_sb = cpool.tile([128, n_chunks], bf)
            nc.sync.dma_start(out=meta_sb[:], in_=meta_ext[:])
            wbf_sb = cpool.tile([128, WBFW], bf)
            nc.sync.dma_start(out=wbf_sb[:], in_=wbf_ext[:])
            fb_sb = cpool.tile([128, FBW], f32)
            nc.sync.dma_start(out=fb_sb[:], in_=fb_ext[:])
            xt_sb = cpool.tile([128, VPC], bf)
            nc.sync.dma_start(out=xt_sb[:], in_=xt_ext[:])
            h1t_sb = cpool.tile([128, VPC], bf)

            iota = wbf_sb[:, IOTA:IOTA + 128]
            ident = wbf_sb[:, IDENT:IDENT + 128]
            ones_row = wbf_sb[0:1, ONESROW:ONESROW + 128]

            crit_acc = cpool.tile([128, 1], f32)
            nc.vector.memset(crit_acc[:], 0.0)

            y_local = dram.tile([VPC, D], bf)
            y_full0 = dram.tile([NPAD, D], bf, addr_space="Shared")
            y_full1 = dram.tile([NPAD, D], bf, addr_space="Shared")

            qctr = [0]
            for L in range(2):
                y_full = y_full0 if L == 0 else y_full1
                srcT = xt_sb if L == 0 else h1t_sb
                wl = wbf_sb[:, (W1L if L == 0 else W2L):(W1L if L == 0 else W2L) + 128]
                wr = wbf_sb[:, (W1R if L == 0 else W2R):(W1R if L == 0 else W2R) + 128]
                b_row = wbf_sb[0:1, (B1ROW if L == 0 else B2ROW):(B1ROW if L == 0 else B2ROW) + 128]

                # ---- pre-transform (L1's is fused into L0's finalize below)
                if L == 0:
                    for t in range(TPC):
                        yp = mm_psum.tile([128, 128], f32, tag="mm", name=f"yp_{L}_{t}")
                        nc.tensor.matmul(
                            out=yp[:], lhsT=srcT[:, t * 128:(t + 1) * 128], rhs=wl,
                            start=True, stop=True,
                        )
                        yl_sb = hpool.tile([128, 128], bf, tag="ylsb", name=f"ylsb_{L}_{t}")
                        nc.scalar.copy(out=yl_sb[:], in_=yp[:])
                        nc.sync.dma_start(out=y_local[t * 128:(t + 1) * 128, :], in_=yl_sb[:])

                nc.gpsimd.collective_compute(
                    "AllGather",
                    mybir.AluOpType.bypass,
                    replica_groups=[list(range(NCORES))],
                    ins=[y_local.opt()],
                    outs=[y_full.opt()],
                )

                # ---- aggregation + finalize, per destination-tile block
                for bi, blk in enumerate(sched["blocks"]):
                    aggs = {}
                    for t in blk:
                        aggs[t] = agg_psum.tile([128, 128], f32, tag="agg", name=f"agg_{L}_{t}")
                    for ci, call in enumerate(sched["calls_by_blk"][bi]):
                        g = gpool.tile([128, max_nch, 128], bf, tag="g", name=f"g_{L}_{bi}_{call.icol}")
                        nidx = call.nch * 128
                        nc.gpsimd.dma_gather(
                            out_ap=g[:, :call.nch, :],
                            in_ap=y_full[call.s * SRCW:(call.s + 1) * SRCW, :],
                            idxs_ap=gidx_sb[:, call.icol: call.icol + nidx // 16],
                            num_idxs=nidx,
                            num_idxs_reg=nidx,
                            elem_size=D,
                            single_packet=False,
                            queue_num=qctr[0] % 4,
                        )
                        qctr[0] += 1
                        npair = len(call.chunks)
                        for j0 in range(0, npair, OHG):
                            gn = min(OHG, npair - j0)
                            pc0 = call.chunks[j0][1]
                            ohg = ohpool.tile([128, OHG * 128], bf, tag="oh", name=f"oh_{L}_{pc0}")
                            nc.vector.tensor_tensor(
                                out=ohg[:, :gn * 128].rearrange("p (a f) -> p a f", a=gn),
                                in0=iota.rearrange("p (a f) -> p a f", a=1).to_broadcast([128, gn, 128]),
                                in1=meta_sb[:, pc0:pc0 + gn].to_broadcast([128, gn, 128]),
                                op=mybir.AluOpType.is_equal,
                            )
                            for jj in range(gn):
                                t, pc, first, last, jc = call.chunks[j0 + jj]
                                nc.tensor.matmul(
                                    out=aggs[t][:],
                                    lhsT=ohg[:, jj * 128:(jj + 1) * 128],
                                    rhs=g[:, jc: jc + 1, :],
                                    start=first,
                                    stop=last,
                                )

                    for t in blk:
                        tsl = slice(t * 128, (t + 1) * 128)
                        magg = hpool.tile([128, 128], bf, tag="magg", name=f"magg_{L}_{t}")
                        if t in sched["t_zero"]:
                            nc.vector.memset(magg[:], 0.0)
                        else:
                            nc.scalar.activation(
                                out=magg[:], in_=aggs[t][:],
                                func=mybir.ActivationFunctionType.Identity,
                                bias=0.0, scale=fb_sb[:, 2 + t: 3 + t],
                            )
                        hp = mm_psum.tile([128, 128], f32, tag="mm", name=f"hp_{L}_{t}")
                        nc.tensor.matmul(out=hp[:], lhsT=wr, rhs=srcT[:, tsl],
                                         start=True, stop=False)
                        nc.tensor.matmul(out=hp[:], lhsT=b_row, rhs=ones_row,
                                         start=False, stop=False)
                        nc.tensor.matmul(out=hp[:], lhsT=magg[:], rhs=ident,
                                         start=False, stop=True)
                        if L == 0:
                            nc.scalar.activation(
                                out=h1t_sb[:, tsl], in_=hp[:],
                                func=mybir.ActivationFunctionType.Relu,
                            )
                            w2l = wbf_sb[:, W2L:W2L + 128]
                            yp1 = mm_psum.tile([128, 128], f32, tag="mm", name=f"yp1_{t}")
                            nc.tensor.matmul(
                                out=yp1[:], lhsT=h1t_sb[:, tsl],
                                rhs=w2l, start=True, stop=True,
                            )
                            yl1_sb = hpool.tile([128, 128], bf, tag="ylsb", name=f"yl1sb_{t}")
                            nc.scalar.copy(out=yl1_sb[:], in_=yp1[:])
                            nc.sync.dma_start(out=y_local[tsl, :], in_=yl1_sb[:])
                        else:
                            hT = hpool.tile([128, 128], bf, tag="hT", name=f"hT_{t}")
                            rs = hpool.tile([128, 1], f32, tag="rs", name=f"rs_{t}")
                            nc.scalar.activation(
                                out=hT[:], in_=hp[:],
                                func=mybir.ActivationFunctionType.Relu,
                                accum_out=rs[:],
                            )
                            ap_ = mm_psum.tile([1, 128], f32, tag="mm", name=f"act_{t}")
                            nc.tensor.matmul(
                                out=ap_[:], lhsT=wbf_sb[:, WACOL:WACOL + 1], rhs=hT[:],
                                start=True, stop=True,
                            )
                            arow = hpool.tile([1, 128], f32, tag="arow", name=f"arow_{t}")
                            nc.scalar.activation(
                                out=arow[:], in_=ap_[:],
                                func=mybir.ActivationFunctionType.Identity,
                                bias=fb_sb[0:1, 0:1], scale=1.0,
                            )
                            nc.sync.dma_start(
                                out=actor_ext[0:1, tsl], in_=arow[:]
                            )
                            nc.vector.tensor_add(out=crit_acc[:], in0=crit_acc[:], in1=rs[:])

            cp = mm_psum.tile([1, 1], f32, tag="mm", name="critp")
            nc.tensor.matmul(out=cp[:], lhsT=crit_acc[:], rhs=fb_sb[:, 1:2], start=True, stop=True)
            crit_sb = hpool.tile([1, 1], f32, tag="csb", name="crit_sb")
            nc.scalar.copy(out=crit_sb[:], in_=cp[:])
            nc.sync.dma_start(out=crit_ext[:], in_=crit_sb[:])

    nc.finalize()
    return nc


def kernel(x, edge_index, W1_l, b1, W1_r, W2_l, b2, W2_r, Wa, ba, Wc, bc):
    global LAST_RESULT
    x = np.asarray(x)
    assert x.shape == (N, D)

    sched, planes = _build_schedule(np.asarray(edge_index))
    nc = _build_kernel(sched)

    xpad = np.zeros((NPAD, D), dtype=np.float32)
    xpad[:N] = np.asarray(x, np.float32)

    wbf = np.zeros((128, WBFW), dtype=BF16)
    wbf[:, W1L:W1L + 128] = np.asarray(W1_l, np.float32).T.astype(BF16)
    wbf[:, W1R:W1R + 128] = np.asarray(W1_r, np.float32).T.astype(BF16)
    wbf[:, W2L:W2L + 128] = np.asarray(W2_l, np.float32).T.astype(BF16)
    wbf[:, W2R:W2R + 128] = np.asarray(W2_r, np.float32).T.astype(BF16)
    wbf[:, WACOL] = np.asarray(Wa, np.float32)[0].astype(BF16)
    wbf[:, IOTA:IOTA + 128] = np.tile(
        np.arange(128, dtype=np.float32)[None, :], (128, 1)).astype(BF16)
    wbf[:, IDENT:IDENT + 128] = np.eye(128, dtype=np.float32).astype(BF16)
    wbf[0, ONESROW:ONESROW + 128] = np.ones(128, np.float32).astype(BF16)
    wbf[0, B1ROW:B1ROW + 128] = np.asarray(b1, np.float32).astype(BF16)
    wbf[0, B2ROW:B2ROW + 128] = np.asarray(b2, np.float32).astype(BF16)

    in_maps = []
    for p in range(NCORES):
        idx_plane, meta, inv_p = planes[p]
        fb = np.zeros((128, FBW), dtype=np.float32)
        fb[0, 0] = np.float32(np.asarray(ba).reshape(-1)[0])
        fb[:, 1] = np.asarray(Wc, np.float32)[0]
        fb[:, 2:] = inv_p
        xt = np.ascontiguousarray(xpad[p * VPC:(p + 1) * VPC].T).astype(BF16)
        in_maps.append({
            "xt": xt, "gidx": idx_plane, "meta": meta, "wbf": wbf, "fb": fb,
        })

    res = run_bass_kernel_spmd(nc, in_maps, core_ids=list(range(NCORES)))
    LAST_RESULT = res

    actor = np.concatenate([res.results[p]["actor"][0] for p in range(NCORES)])[:N]
    crit_sum = np.sum([res.results[p]["crit"][0, 0] for p in range(NCORES)])
    critic = np.float32(crit_sum / N + np.float32(np.asarray(bc).reshape(-1)[0]))
    return actor.astype(np.float32), critic


# revision 26
# speedup vs baseline: 1.1630x; 1.0042x over previous
"""Distributed SAGE GNN kernel for 8 TRN2 NeuronCores.

Strategy (per sharding hint): nodes and their output rows are sharded across
the 8 cores; edges are partitioned by destination core. Weights replicated.

Per layer:
  1. pre-transform: y = input @ W_l.T computed on each core's node shard
     (feature-major input tiles as matmul lhsT), then one AllGather so every
     core holds the full y (message table) in its HBM.
  2. aggregation: for each 128-node destination tile, gather y[src] rows with
     dma_gather (bf16 rows, int16 window-local indices, 4 source windows of
     25088 rows so indices fit int16; 4 SWDGE queues round-robin) and
     accumulate  agg[n, f] = sum_e onehot[e, n] * y[src_e, f]  as one-hot x
     message matmuls in PSUM. One-hots are built 16 pairs per DVE op via
     broadcast is_equal against an iota tile; 128-edge chunks may straddle
     tile boundaries (one matmul per (chunk, tile) pair, the dst-relative
     meta column masks foreign edges).
  3. finalize: h^T = relu(W_r.T-term + bias + agg/deg) fused into one PSUM
     accumulation group (bias via a K=1 matmul, agg via a transposing matmul
     with lhsT=agg), the 1/deg mean scale applied by the Scalar engine on the
     PSUM->SBUF copy. Layer 2's pre-transform is fused into layer 1's
     finalize; actor/critic heads are tiny matmuls/reductions in the L2 loop.
"""

import numpy as np
import ml_dtypes

import concourse.bass as bass
import concourse.mybir as mybir
import concourse.tile as tile
from concourse import bacc
from concourse.bass_utils import run_bass_kernel_spmd

BF16 = ml_dtypes.bfloat16

N = 100000
E = 1600000
D = 128
NCORES = 8
VPC = 12544            # nodes per core (padded)
NPAD = VPC * NCORES    # 100352
TPC = VPC // 128       # 98 dst tiles per core
NSRC = 4               # gather source windows (int16 index limit)
SRCW = NPAD // NSRC    # 25088
TB = 3                 # dst tiles per block (PSUM-resident accumulators)
MAXCH = 32             # max 128-edge chunks per dma_gather call
OHG = 32               # pairs per batched one-hot DVE op

# wbf plane column layout (bf16)
W1L, W1R, W2L, W2R = 0, 128, 256, 384
WACOL = 512
IOTA, IDENT = 514, 642
ONESROW, B1ROW, B2ROW = 770, 898, 1026
WBFW = 1154
# fb plane (f32): [0,0]=ba, col1 = Wc, cols 2.. = 1/deg per tile
FBW = 2 + TPC

LAST_RESULT = None     # BassKernelResults of the most recent run (for tests)


class _Call:
    __slots__ = ("s", "nch", "icol", "chunks", "tailpad")

    def __init__(self, s, icol):
        self.s = s
        self.nch = 0
        self.icol = icol      # column offset into the idx plane
        self.chunks = []      # list of (t, pair_col, first, last, chunk_in_call)
        self.tailpad = 0      # trailing pad slots (gather row 0, masked)


def _build_schedule(edge_index):
    """Host-side edge partitioning. Returns (sched, per-core planes).

    Edge stream layout per core (identical shape on every core): for each
    block of TB dst tiles, for each source window s, the edges of groups
    (s, t in blk) are packed back-to-back, each group padded to the max count
    over cores (pads: idx 0, sentinel dst). The packed segment is rounded up
    to a multiple of 128. 128-edge chunks may straddle tile boundaries; each
    (chunk, tile) pair becomes one one-hot matmul whose dst-relative meta
    column masks out the other tiles' edges.
    """
    src = np.asarray(edge_index[0], dtype=np.int64)
    dst = np.asarray(edge_index[1], dtype=np.int64)

    core = dst // VPC
    t_loc = (dst - core * VPC) // 128
    s_chunk = src // SRCW
    grp = (core * NSRC + s_chunk) * TPC + t_loc          # [E]

    cnt = np.bincount(grp, minlength=NCORES * NSRC * TPC)
    cnt = cnt.reshape(NCORES, NSRC, TPC)
    C = cnt.max(axis=0)                                  # exact per (s, t)

    deg = np.bincount(dst, minlength=NPAD).astype(np.float64)
    invdeg = (1.0 / np.maximum(deg, 1.0)).astype(np.float32)   # [NPAD]

    blocks = [list(range(b, min(b + TB, TPC))) for b in range(0, TPC, TB)]

    slot_base = np.zeros((NSRC, TPC), dtype=np.int64)
    seg_meta = []            # per (blk, s): (slot0, seg_len, pad_len, groups)
    off = 0
    for blk in blocks:
        for s in range(NSRC):
            g0 = off
            groups = []
            for t in blk:
                slot_base[s, t] = off
                groups.append((t, off - g0, int(C[s, t])))
                off += int(C[s, t])
            seg_len = off - g0
            pad = (-seg_len) % 128
            off += pad
            seg_meta.append((g0, seg_len, pad, groups))
    etot = off
    assert etot % 128 == 0

    # total (chunk, tile) pairs per tile, for start/stop flags
    t_total = np.zeros(TPC, dtype=np.int64)
    for (g0, seg_len, pad, groups) in seg_meta:
        nch_seg = (seg_len + pad) // 128
        for k in range(nch_seg):
            a, b = k * 128, k * 128 + 128
            for (t, goff, glen) in groups:
                if glen and goff < b and a < goff + glen:
                    t_total[t] += 1

    calls_by_blk = [[] for _ in blocks]
    t_seen = np.zeros(TPC, dtype=np.int64)
    pc = 0
    icol = 0
    seg_i = 0
    pair_info = []               # (slot0_of_chunk, t) per pair
    for bi, blk in enumerate(blocks):
        calls = calls_by_blk[bi]
        for s in range(NSRC):
            g0, seg_len, pad, groups = seg_meta[seg_i]
            seg_i += 1
            nch_seg = (seg_len + pad) // 128
            call = None
            for k in range(nch_seg):
                if call is None or call.nch == MAXCH:
                    if call is not None:
                        calls.append(call)
                    call = _Call(s, icol)
                a, b = k * 128, k * 128 + 128
                j_in_call = call.nch
                for (t, goff, glen) in groups:
                    if glen and goff < b and a < goff + glen:
                        first = t_seen[t] == 0
                        t_seen[t] += 1
                        last = t_seen[t] == t_total[t]
                        call.chunks.append((t, pc, bool(first), bool(last), j_in_call))
                        pair_info.append((g0 + a, t))
                        pc += 1
                call.nch += 1
                icol += 8
            call.tailpad = pad
            calls.append(call)
            call = None
    n_pairs = pc
    idx_cols = icol

    # ---- per-core slot arrays -------------------------------------------
    order = np.argsort(grp, kind="stable")
    sg = grp[order]
    n_groups = NCORES * NSRC * TPC
    gstart = np.searchsorted(sg, np.arange(n_groups))
    rank = np.arange(E, dtype=np.int64) - gstart[sg]
    s_o = s_chunk[order]
    t_o = t_loc[order]
    core_o = core[order]
    src_o = src[order]
    dst_o = dst[order]
    slotpos = slot_base[s_o, t_o] + rank

    planes = []
    for p in range(NCORES):
        m = core_o == p
        sl = slotpos[m]
        s_src = np.zeros(etot, dtype=np.int16)
        s_dst = np.full(etot, 999.0, dtype=np.float32)
        s_tile = np.full(etot, -1, dtype=np.int64)
        s_src[sl] = (src_o[m] - s_o[m] * SRCW).astype(np.int16)
        s_dst[sl] = (dst_o[m] - (p * VPC + t_o[m] * 128)).astype(np.float32)
        s_tile[sl] = t_o[m]

        idx_plane = np.zeros((16, idx_cols), dtype=np.int16)
        meta = np.full((128, n_pairs), 999.0, dtype=np.float32)
        for calls in calls_by_blk:
            for call in calls:
                nidx = call.nch * 128
                base_slot = None
                for (t, pcx, first, last, j_in_call) in call.chunks:
                    if base_slot is None:
                        base_slot = pair_info[pcx][0] - j_in_call * 128
                    a = pair_info[pcx][0]
                    col = s_dst[a:a + 128].copy()
                    col[s_tile[a:a + 128] != t] = 999.0
                    meta[:, pcx] = col
                arr = s_src[base_slot: base_slot + nidx]
                idx_plane[:, call.icol: call.icol + nidx // 16] = (
                    arr.reshape(-1, 16).T
                )
        idx_plane = np.tile(idx_plane, (8, 1))

        inv_p = invdeg[p * VPC:(p + 1) * VPC].reshape(TPC, 128).T  # [128, TPC]
        planes.append((idx_plane, meta.astype(BF16), np.ascontiguousarray(inv_p)))

    sched = {
        "t_zero": {int(t) for t in range(TPC) if t_total[t] == 0},
        "blocks": blocks,
        "calls_by_blk": calls_by_blk,
        "n_chunks": n_pairs,
        "idx_cols": idx_cols,
        "etot": etot,
        "max_nch": max(c.nch for calls in calls_by_blk for c in calls),
    }
    return sched, planes


def _build_kernel(sched):
    nc = bacc.Bacc(num_devices=NCORES, num_swdge_queues=4)
    bf = mybir.dt.bfloat16
    f32 = mybir.dt.float32
    n_chunks = sched["n_chunks"]
    idx_cols = sched["idx_cols"]
    max_nch = sched["max_nch"]

    xt_ext = nc.declare_dram_parameter("xt", [D, VPC], bf, isOutput=False)
    gidx_ext = nc.declare_dram_parameter("gidx", [128, idx_cols], mybir.dt.int16, isOutput=False)
    meta_ext = nc.declare_dram_parameter("meta", [128, n_chunks], bf, isOutput=False)
    wbf_ext = nc.declare_dram_parameter("wbf", [128, WBFW], bf, isOutput=False)
    fb_ext = nc.declare_dram_parameter("fb", [128, FBW], f32, isOutput=False)
    actor_ext = nc.declare_dram_parameter("actor", [1, VPC], f32, isOutput=True)
    crit_ext = nc.declare_dram_parameter("crit", [1, 1], f32, isOutput=True)

    with tile.TileContext(nc) as tc:
        with (
            tc.tile_pool(name="cpool", bufs=1) as cpool,
            tc.tile_pool(name="gpool", bufs=12) as gpool,
            tc.tile_pool(name="ohpool", bufs=6) as ohpool,
            tc.tile_pool(name="hpool", bufs=6) as hpool,
            tc.tile_pool(name="agg_psum", bufs=6, space="PSUM") as agg_psum,
            tc.tile_pool(name="mm_psum", bufs=2, space="PSUM") as mm_psum,
            tc.tile_pool(name="dram", bufs=1, space="DRAM") as dram,
        ):
            gidx_sb = cpool.tile([128, idx_cols], mybir.dt.int16)
            nc.sync.dma_start(out=gidx_sb[:], in_=gidx_ext[:])
            meta_sb = cpool.tile([128, n_chunks], bf)
            nc.sync.dma_start(out=meta_sb[:], in_=meta_ext[:])
            wbf_sb = cpool.tile([128, WBFW], bf)
            nc.sync.dma_start(out=wbf_sb[:], in_=wbf_ext[:])
            fb_sb = cpool.tile([128, FBW], f32)
            nc.sync.dma_start(out=fb_sb[:], in_=fb_ext[:])
            xt_sb = cpool.tile([128, VPC], bf)
            nc.sync.dma_start(out=xt_sb[:], in_=xt_ext[:])
            h1t_sb = cpool.tile([128, VPC], bf)

            iota = wbf_sb[:, IOTA:IOTA + 128]
            ident = wbf_sb[:, IDENT:IDENT + 128]
            ones_row = wbf_sb[0:1, ONESROW:ONESROW + 128]

            crit_acc = cpool.tile([128, 1], f32)
            nc.vector.memset(crit_acc[:], 0.0)

            y_local = dram.tile([VPC, D], bf)
            y_full0 = dram.tile([NPAD, D], bf, addr_space="Shared")
            y_full1 = dram.tile([NPAD, D], bf, addr_space="Shared")

            qctr = [0]
            for L in range(2):
                y_full = y_full0 if L == 0 else y_full1
                srcT = xt_sb if L == 0 else h1t_sb
                wl = wbf_sb[:, (W1L if L == 0 else W2L):(W1L if L == 0 else W2L) + 128]
                wr = wbf_sb[:, (W1R if L == 0 else W2R):(W1R if L == 0 else W2R) + 128]
                b_row = wbf_sb[0:1, (B1ROW if L == 0 else B2ROW):(B1ROW if L == 0 else B2ROW) + 128]

                # ---- pre-transform (L1's is fused into L0's finalize below)
                if L == 0:
                    for t in range(TPC):
                        yp = mm_psum.tile([128, 128], f32, tag="mm", name=f"yp_{L}_{t}")
                        nc.tensor.matmul(
                            out=yp[:], lhsT=srcT[:, t * 128:(t + 1) * 128], rhs=wl,
                            start=True, stop=True,
                        )
                        yl_sb = hpool.tile([128, 128], bf, tag="ylsb", name=f"ylsb_{L}_{t}")
                        nc.scalar.copy(out=yl_sb[:], in_=yp[:])
                        nc.sync.dma_start(out=y_local[t * 128:(t + 1) * 128, :], in_=yl_sb[:])

                nc.gpsimd.collective_compute(
                    "AllGather",
                    mybir.AluOpType.bypass,
                    replica_groups=[list(range(NCORES))],
                    ins=[y_local.opt()],
                    outs=[y_full.opt()],
                )

                # ---- aggregation + finalize, per destination-tile block
                for bi, blk in enumerate(sched["blocks"]):
                    aggs = {}
                    for t in blk:
                        aggs[t] = agg_psum.tile([128, 128], f32, tag="agg", name=f"agg_{L}_{t}")
                    for ci, call in enumerate(sched["calls_by_blk"][bi]):
                        g = gpool.tile([128, max_nch, 128], bf, tag="g", name=f"g_{L}_{bi}_{call.icol}")
                        nidx = call.nch * 128
                        nc.gpsimd.dma_gather(
                            out_ap=g[:, :call.nch, :],
                            in_ap=y_full[call.s * SRCW:(call.s + 1) * SRCW, :],
                            idxs_ap=gidx_sb[:, call.icol: call.icol + nidx // 16],
                            num_idxs=nidx,
                            num_idxs_reg=nidx,
                            elem_size=D,
                            single_packet=False,
                            queue_num=qctr[0] % 4,
                        )
                        qctr[0] += 1
                        npair = len(call.chunks)
                        for j0 in range(0, npair, OHG):
                            gn = min(OHG, npair - j0)
                            pc0 = call.chunks[j0][1]
                            ohg = ohpool.tile([128, OHG * 128], bf, tag="oh", name=f"oh_{L}_{pc0}")
                            nc.vector.tensor_tensor(
                                out=ohg[:, :gn * 128].rearrange("p (a f) -> p a f", a=gn),
                                in0=iota.rearrange("p (a f) -> p a f", a=1).to_broadcast([128, gn, 128]),
                                in1=meta_sb[:, pc0:pc0 + gn].to_broadcast([128, gn, 128]),
                                op=mybir.AluOpType.is_equal,
                            )
                            for jj in range(gn):
                                t, pc, first, last, jc = call.chunks[j0 + jj]
                                nc.tensor.matmul(
                                    out=aggs[t][:],
                                    lhsT=ohg[:, jj * 128:(jj + 1) * 128],
                                    rhs=g[:, jc: jc + 1, :],
                                    start=first,
                                    stop=last,
                                )

                    for t in blk:
                        tsl = slice(t * 128, (t + 1) * 128)
                        magg = hpool.tile([128, 128], bf, tag="magg", name=f"magg_{L}_{t}")
                        if t in sched["t_zero"]:
                            nc.vector.memset(magg[:], 0.0)
                        else:
                            nc.scalar.activation(
                                out=magg[:], in_=aggs[t][:],
                                func=mybir.ActivationFunctionType.Identity,
                                bias=0.0, scale=fb_sb[:, 2 + t: 3 + t],
                            )
                        hp = mm_psum.tile([128, 128], f32, tag="mm", name=f"hp_{L}_{t}")
                        nc.tensor.matmul(out=hp[:], lhsT=wr, rhs=srcT[:, tsl],
                                         start=True, stop=False)
                        nc.tensor.matmul(out=hp[:], lhsT=b_row, rhs=ones_row,
                                         start=False, stop=False)
                        nc.tensor.matmul(out=hp[:], lhsT=magg[:], rhs=ident,
                                         start=False, stop=True)
                        if L == 0:
                            nc.scalar.activation(
                                out=h1t_sb[:, tsl], in_=hp[:],
                                func=mybir.ActivationFunctionType.Relu,
                            )
                            w2l = wbf_sb[:, W2L:W2L + 128]
                            yp1 = mm_psum.tile([128, 128], f32, tag="mm", name=f"yp1_{t}")
                            nc.tensor.matmul(
                                out=yp1[:], lhsT=h1t_sb[:, tsl],
                                rhs=w2l, start=True, stop=True,
                            )
                            yl1_sb = hpool.tile([128, 128], bf, tag="ylsb", name=f"yl1sb_{t}")
                            nc.scalar.copy(out=yl1_sb[:], in_=yp1[:])
                            nc.sync.dma_start(out=y_local[tsl, :], in_=yl1_sb[:])
                        else:
                            hT = hpool.tile([128, 128], bf, tag="hT", name=f"hT_{t}")
                            rs = hpool.tile([128, 1], f32, tag="rs", name=f"rs_{t}")
                            nc.scalar.activation(
                                out=hT[:], in_=hp[:],
                                func=mybir.ActivationFunctionType.Relu,
                                accum_out=rs[:],
                            )
                            ap_ = mm_psum.tile([1, 128], f32, tag="mm", name=f"act_{t}")
                            nc.tensor.matmul(
                                out=ap_[:], lhsT=wbf_sb[:, WACOL:WACOL + 1], rhs=hT[:],
                                start=True, stop=True,
                            )
                            arow = hpool.tile([1, 128], f32, tag="arow", name=f"arow_{t}")
                            nc.scalar.activation(
                                out=arow[:], in_=ap_[:],
                                func=mybir.ActivationFunctionType.Identity,
                                bias=fb_sb[0:1, 0:1], scale=1.0,
                            )
                            nc.sync.dma_start(
                                out=actor_ext[0:1, tsl], in_=arow[:]
                            )
                            nc.vector.tensor_add(out=crit_acc[:], in0=crit_acc[:], in1=rs[:])

            cp = mm_psum.tile([1, 1], f32, tag="mm", name="critp")
            nc.tensor.matmul(out=cp[:], lhsT=crit_acc[:], rhs=fb_sb[:, 1:2], start=True, stop=True)
            crit_sb = hpool.tile([1, 1], f32, tag="csb", name="crit_sb")
            nc.scalar.copy(out=crit_sb[:], in_=cp[:])
            nc.sync.dma_start(out=crit_ext[:], in_=crit_sb[:])

    nc.finalize()
    return nc


def kernel(x, edge_index, W1_l, b1, W1_r, W2_l, b2, W2_r, Wa, ba, Wc, bc):
    global LAST_RESULT
    x = np.asarray(x)
    assert x.shape == (N, D)

    sched, planes = _build_schedule(np.asarray(edge_index))
    nc = _build_kernel(sched)

    xpad = np.zeros((NPAD, D), dtype=np.float32)
    xpad[:N] = np.asarray(x, np.float32)

    wbf = np.zeros((128, WBFW), dtype=BF16)
    wbf[:, W1L:W1L + 128] = np.asarray(W1_l, np.float32).T.astype(BF16)
    wbf[:, W1R:W1R + 128] = np.asarray(W1_r, np.float32).T.astype(BF16)
    wbf[:, W2L:W2L + 128] = np.asarray(W2_l, np.float32).T.astype(BF16)
    wbf[:, W2R:W2R + 128] = np.asarray(W2_r, np.float32).T.astype(BF16)
    wbf[:, WACOL] = np.asarray(Wa, np.float32)[0].astype(BF16)
    wbf[:, IOTA:IOTA + 128] = np.tile(
        np.arange(128, dtype=np.float32)[None, :], (128, 1)).astype(BF16)
    wbf[:, IDENT:IDENT + 128] = np.eye(128, dtype=np.float32).astype(BF16)
    wbf[0, ONESROW:ONESROW + 128] = np.ones(128, np.float32).astype(BF16)
    wbf[0, B1ROW:B1ROW + 128] = np.asarray(b1, np.float32).astype(BF16)
    wbf[0, B2ROW:B2ROW + 128] = np.asarray(b2, np.float32).astype(BF16)

    in_maps = []
    for p in range(NCORES):
        idx_plane, meta, inv_p = planes[p]
        fb = np.zeros((128, FBW), dtype=np.float32)
        fb[0, 0] = np.float32(np.asarray(ba).reshape(-1)[0])
        fb[:, 1] = np.asarray(Wc, np.float32)[0]
        fb[:, 2:] = inv_p
        xt = np.ascontiguousarray(xpad[p * VPC:(p + 1) * VPC].T).astype(BF16)
        in_maps.append({
            "xt": xt, "gidx": idx_plane, "meta": meta, "wbf": wbf, "fb": fb,
        })

    res = run_bass_kernel_spmd(nc, in_maps, core_ids=list(range(NCORES)))
    LAST_RESULT = res

    actor = np.concatenate([res.results[p]["actor"][0] for p in range(NCORES)])[:N]
    crit_sum = np.sum([res.results[p]["crit"][0, 0] for p in range(NCORES)])
    critic = np.float32(crit_sum / N + np.float32(np.asarray(bc).reshape(-1)[0]))
    return actor.astype(np.float32), critic
